# revision 8
# baseline (speedup 1.0000x reference)
"""Multi-head self-attention (B=2, T=2048, C=1024, H=16) on 8 TRN2 NeuronCores.

Sharding: tensor-parallel over heads. Core m owns heads (2m, 2m+1):
  - computes qkv^T = (Wqkv_shard^T) @ x^T for its 2 heads (contraction-major
    layouts; host pre-transposes x so no on-chip transposes of x are needed)
  - causal attention for its 2 heads (both batches), flash-style with
    blockwise exp (no max-subtraction: scores are O(1) here) and a
    ones-column matmul that produces the softmax denominator for free
  - partial output projection partial_m = values_m @ Wo[rows of heads m]
Host sums the 8 partials and adds biases bo.
"""

import numpy as np

import concourse.bass as bass
import concourse.bacc as bacc
import concourse.mybir as mybir
import concourse.tile as tile
from concourse.bass_utils import run_bass_kernel_spmd

B, T, C = 2, 2048, 1024
H, HS = 16, 64
N_CORES = 8
HPC = H // N_CORES            # heads per core = 2
R = B * T                      # 4096 rows total
IC_W = 512                     # i-chunk width (query cols per block)
JT_W = 128                     # j-tile width (key rows per block)
N_IC = T // IC_W               # 4 i-chunks per batch
N_JT = T // JT_W               # 16 j-tiles per batch
F32 = mybir.dt.float32
F32R = mybir.dt.float32r

# compute dtype for matmuls: float32r runs the PE at full rate (1 cyc/row for
# free-dim >= 256) with near-fp32 storage; plain float32 is 4x slower.
USE_F32R = True


def _mm(d):
    return d


def _build(causal: bool):
    nc = bacc.Bacc("TRN2", target_bir_lowering=False, debug=False,
                   num_devices=N_CORES)

    xt_d = nc.dram_tensor("xt", [C, R], F32R, kind="ExternalInput")
    w3_d = nc.dram_tensor("w3", [C, 3 * 128], F32R, kind="ExternalInput")
    b3_d = nc.dram_tensor("b3", [3, 128, 1], F32, kind="ExternalInput")
    wo_d = nc.dram_tensor("wo", [128, C], F32R, kind="ExternalInput")
    msk_d = nc.dram_tensor("msk", [4, 128, IC_W], F32R, kind="ExternalInput")
    idn_d = nc.dram_tensor("idn", [128, 64], F32R, kind="ExternalInput")
    one_d = nc.dram_tensor("ones", [128, 64], F32R, kind="ExternalInput")
    out_d = nc.dram_tensor("part", [R, C], F32, kind="ExternalOutput")

    with tile.TileContext(nc) as tc:
        with (
            tc.tile_pool(name="const", bufs=1) as cpool,
            tc.tile_pool(name="qkv", bufs=1) as qkvpool,
            tc.tile_pool(name="xt", bufs=16) as xtpool,
            tc.tile_pool(name="pt", bufs=4) as ptpool,
            tc.tile_pool(name="small", bufs=4) as smallpool,
            tc.tile_pool(name="ot", bufs=2) as otpool,
            tc.tile_pool(name="osb", bufs=4) as opool,
            tc.tile_pool(name="ps_mm", bufs=2, space="PSUM") as ps_mm,
            tc.tile_pool(name="ps_s", bufs=2, space="PSUM") as ps_s,
            tc.tile_pool(name="ps_o", bufs=2, space="PSUM") as ps_o,
            tc.tile_pool(name="ps_cs", bufs=2, space="PSUM") as ps_cs,
        ):
            # ---- constants ----
            w3_sb = []
            for ct in range(8):
                t_ = cpool.tile([128, 384], F32R, tag=f"w3_{ct}")
                nc.sync.dma_start(t_[:], w3_d.ap()[128 * ct:128 * (ct + 1), :])
                w3_sb.append(t_)
            bias_sb = []
            for n in range(3):
                t_ = cpool.tile([128, 1], F32, tag=f"b3_{n}")
                nc.sync.dma_start(t_[:], b3_d.ap()[n])
                bias_sb.append(t_)
            wo_sb = cpool.tile([128, C], F32R, tag="wo")
            nc.sync.dma_start(wo_sb[:], wo_d.ap()[:])
            msk_sb = []
            for r_ in range(4):
                t_ = cpool.tile([128, IC_W], F32R, tag=f"msk_{r_}")
                nc.sync.dma_start(t_[:], msk_d.ap()[r_])
                msk_sb.append(t_)
            idn_sb = cpool.tile([128, 64], F32R, tag="idn")
            nc.sync.dma_start(idn_sb[:], idn_d.ap()[:])
            ones_sb = cpool.tile([128, 64], F32R, tag="ones_sb")
            nc.sync.dma_start(ones_sb[:], one_d.ap()[:])


            # persistent qkv^T (transposed layouts, heads packed 2-up)
            qt2 = qkvpool.tile([128, R], F32R, tag="qt2")  # rows 0:64 h0, 64:128 h1
            kt2 = qkvpool.tile([128, R], F32R, tag="kt2")
            vt2 = qkvpool.tile([128, R], F32R, tag="vt2")
            # v in natural [key, dim] layout per (b, h): [128, 16*64]
            vn_sb = {}
            for b in range(B):
                for h in range(HPC):
                    vn_sb[(b, h)] = qkvpool.tile([128, N_JT * HS], F32R,
                                                 name=f"vn_{b}_{h}",
                                                 tag=f"vn_{b}_{h}")

            for b in range(B):
                # ---- QKV projection for this batch's 4 i-chunks ----
                for icl in range(4):
                    icg = 4 * b + icl
                    i0 = IC_W * icg
                    xts = []
                    for ct in range(8):
                        t_ = xtpool.tile([128, IC_W], F32R)
                        nc.sync.dma_start(
                            t_[:], xt_d.ap()[128 * ct:128 * (ct + 1),
                                             i0:i0 + IC_W])
                        xts.append(t_)
                    for n, dst in enumerate((qt2, kt2, vt2)):
                        ps = ps_mm.tile([128, IC_W], F32, tag="mm")
                        for ct in range(8):
                            nc.tensor.matmul(
                                ps[:],
                                _mm(w3_sb[ct][:, 128 * n:128 * (n + 1)]),
                                _mm(xts[ct][:]),
                                start=(ct == 0), stop=(ct == 7))
                        nc.vector.tensor_scalar_add(
                            dst[:, i0:i0 + IC_W], ps[:], bias_sb[n][:])

                # ---- v natural layout (transpose v^T tiles) ----
                for h in range(HPC):
                    for jt in range(N_JT):
                        j0 = T * b + JT_W * jt
                        psv = ps_mm.tile([128, IC_W], F32R, tag="mm")
                        nc.tensor.transpose(
                            psv[:, 0:HS],
                            vt2[64 * h:64 * (h + 1), j0:j0 + JT_W],
                            idn_sb[64 * h:64 * (h + 1), 0:64])
                        nc.vector.tensor_copy(
                            vn_sb[(b, h)][:, HS * jt:HS * (jt + 1)],
                            psv[:, 0:HS])

                # ---- attention + projection per i-chunk ----
                for icl in range(4):
                    i0 = T * b + IC_W * icl
                    njt = 4 * icl + 4 if causal else N_JT
                    pso = [ps_o.tile([128, IC_W], F32, tag="o", name=f"pso_{h_}")
                           for h_ in range(HPC)]
                    pscs = [ps_cs.tile([64, IC_W], F32, tag="cs", name=f"pscs_{h_}")
                            for h_ in range(HPC)]
                    for jt in range(njt):
                        j0 = T * b + JT_W * jt
                        for h in range(HPC):
                            h0 = 64 * h
                            pss = ps_s.tile([128, IC_W], F32, tag="s")
                            nc.tensor.matmul(
                                pss[:],
                                _mm(kt2[h0:h0 + 64, j0:j0 + JT_W]),
                                _mm(qt2[h0:h0 + 64, i0:i0 + IC_W]),
                                start=True, stop=True,
                                tile_position=(h0, 0))
                            pt = ptpool.tile([128, IC_W], F32R, tag="pt")
                            nc.scalar.activation(
                                pt[:], pss[:],
                                mybir.ActivationFunctionType.Exp)
                            if causal:
                                r_ = jt - 4 * icl
                                if r_ >= 0:
                                    nc.vector.tensor_mul(
                                        pt[:], pt[:], msk_sb[r_][:])
                            nc.tensor.matmul(
                                pso[h][0:64, :],
                                _mm(vn_sb[(b, h)][:, HS * jt:HS * (jt + 1)]),
                                _mm(pt[:]),
                                start=(jt == 0), stop=(jt == njt - 1),
                                tile_position=(0, 0), skip_group_check=True)
                            nc.tensor.matmul(
                                pscs[h][:],
                                _mm(ones_sb[:, 0:64]),
                                _mm(pt[:]),
                                start=(jt == 0), stop=(jt == njt - 1),
                                tile_position=(0, 0), skip_group_check=True)
                    # normalize -> ot [128, 512] (h0 rows 0:64, h1 rows 64:128)
                    ot = otpool.tile([128, IC_W], F32R, tag="ot")
                    for h in range(HPC):
                        rcb = smallpool.tile([64, IC_W], F32, tag="rcb")
                        nc.vector.reciprocal(rcb[:], pscs[h][:])
                        nc.vector.tensor_mul(
                            ot[64 * h:64 * (h + 1), :], pso[h][0:64, :],
                            rcb[:])
                    # projection: partial[i0:i0+512, :] = ot^T @ wo
                    for it in range(4):
                        for oc in range(2):
                            psp = ps_mm.tile([128, IC_W], F32, tag="mm")
                            nc.tensor.matmul(
                                psp[:],
                                _mm(ot[:, 128 * it:128 * (it + 1)]),
                                _mm(wo_sb[:, IC_W * oc:IC_W * (oc + 1)]),
                                start=True, stop=True)
                            osb = opool.tile([128, IC_W], F32, tag="osb")
                            nc.vector.tensor_copy(osb[:], psp[:])
                            r0 = i0 + 128 * it
                            nc.sync.dma_start(
                                out_d.ap()[r0:r0 + 128,
                                           IC_W * oc:IC_W * (oc + 1)],
                                osb[:])
    nc.compile()
    return nc


_PROGS = {}


def _get_prog(causal: bool):
    if causal not in _PROGS:
        _PROGS[causal] = _build(causal)
    return _PROGS[causal]


def _prep_inputs(x, Wqkv, bqkv, Wo):
    """Per-core input maps (host-side sharding)."""
    x = np.asarray(x, dtype=np.float32)
    Wqkv = np.asarray(Wqkv, dtype=np.float32)
    bqkv = np.asarray(bqkv, dtype=np.float32)
    Wo = np.asarray(Wo, dtype=np.float32)

    xt = np.ascontiguousarray(x.reshape(R, C).T)  # [C, R]

    # causal mask tiles for the 4 diagonal block offsets
    jl = np.arange(JT_W)[:, None]
    il = np.arange(IC_W)[None, :]
    msk = np.stack([(JT_W * r_ + jl <= il) for r_ in range(4)]).astype(np.float32)
    idn = np.tile(np.eye(64, dtype=np.float32), (2, 1))

    in_maps = []
    scale = 1.0 / np.sqrt(np.float32(HS))
    for m in range(N_CORES):
        h0, h1 = HPC * m, HPC * m + 1
        cols = {}
        for name, off, sc in (("q", 0, scale), ("k", HS, 1.0), ("v", 2 * HS, 1.0)):
            blk = [Wqkv[:, 192 * h + off:192 * h + off + HS] * sc
                   for h in (h0, h1)]
            bb = [bqkv[192 * h + off:192 * h + off + HS] * sc for h in (h0, h1)]
            cols[name] = (np.concatenate(blk, axis=1),
                          np.concatenate(bb))
        w3 = np.concatenate([cols["q"][0], cols["k"][0], cols["v"][0]], axis=1)
        b3 = np.stack([cols["q"][1], cols["k"][1], cols["v"][1]])[..., None]
        wo = Wo[128 * m:128 * (m + 1), :]
        in_maps.append({
            "xt": np.ascontiguousarray(xt),
            "w3": np.ascontiguousarray(w3.astype(np.float32)),
            "b3": np.ascontiguousarray(b3.astype(np.float32)),
            "wo": np.ascontiguousarray(wo.astype(np.float32)),
            "msk": msk,
            "idn": idn,
            "ones": np.ones((128, 64), dtype=np.float32),
        })
    return in_maps


def kernel(x, Wqkv, bqkv, Wo, bo, mask):
    causal = bool(np.asarray(mask).item()) if not isinstance(mask, (int, bool)) else bool(mask)
    nc = _get_prog(causal)
    in_maps = _prep_inputs(x, Wqkv, bqkv, Wo)
    res = run_bass_kernel_spmd(nc, in_maps, list(range(N_CORES)))
    acc = np.zeros((R, C), dtype=np.float32)
    for m in range(N_CORES):
        acc += res.results[m]["part"]
    acc += np.asarray(bo, dtype=np.float32)[None, :]
    return acc.reshape(B, T, C)
